# revision 22
# baseline (speedup 1.0000x reference)
"""RNN-T joint network kernel for Trainium2 (8 NeuronCores, data-parallel over B).

Computes logits = relu(f @ W1f.T + g @ W1g.T + b1) @ W2.T + b2 over the
(B, T, U, ...) broadcast grid without materializing the concat tensor.

Division of labor:
  - Host (cheap, 1.1% of FLOPs, exact fp32 BLAS): the first-layer
    projections pf = f @ W1f.T and pg = g @ W1g.T + b1, shipped to the
    device as fp16 -- 300 KB per core instead of f/g/W1 (1.6 MB).
  - Device (98.9% of FLOPs): the (B,T,U) broadcast join
    h = relu(pf[t] + pg[u]) and the big second-layer matmul h @ W2q.T,
    W2 pre-scaled by QSCALE so the PSUM result is already in int8 units.

Per core (one batch element), grid flattened u-major: g = u*T + t:
  - For each 2048-point span: hT[jc][:, :] = relu(pfT[jc][:, t-slice] +
    pgT_b1[jc][:, u]) (DVE tensor_scalar, fused add+max, fp16, segments
    break only at u boundaries -> few large instructions).
  - Second matmul with h *stationary* and W2q moving, so PSUM comes out
    grid-major: pt[g 128, vocab 1024] += hT[jc][:, gblock].T @ W2q[jc].
  - Drain (DVE scalar_tensor_tensor): int8(pt + QSCALE*b2) -> SBUF ->
    one contiguous 128 KB DMA per grid block into outQ[G, V].
  - Host dequantizes (x * S/127 in fp32) and reshapes; outQ is
    grid-major so the host transpose moves contiguous 1 KB rows.

Quantization: the int8 full-scale starts at S_OUT=2.0, which bounds
max|logits| (~1.57 for the spec'd inputs) with margin, and is adapted
upward per call from a host-side exact sample of ~3k logits if the
inputs ever run hotter. QSCALE is folded into W2/b2 on the host, so
the device program is scale-independent. int8 quantization error is
~0.8% of scale, well inside the 2e-2 relative-error budget.

Dispatch: a lean PJRT shard_map path ships all inputs as ONE fp16 blob
per core and passes 1-element dummies for the donated output operands
(the NEFF binds no input to them; the kernel writes every element of
outQ), so per-call host<->device traffic is ~1.36 MB per core up and
~20.7 MB of int8 logits per core down -- vs ~89 MB/core round-trip for
the naive fp32/zero-buffer path. Falls back to
bass_utils.run_bass_kernel_spmd if anything in the lean path fails,
and uses run_bass_kernel_spmd directly when tracing/profiling kwargs
are requested.
"""

import os
import sys

sys.path.insert(0, "/opt/trn_rl_repo")

import numpy as np

from concourse import bacc, bass, tile, mybir
from concourse.bass_utils import run_bass_kernel_spmd

B, T, U = 8, 200, 101
ENC_H, PRED_H, JH, V = 1024, 320, 512, 1024
G = U * T  # 20200 grid points per core, u-major: g = u*T + t
SPAN = 2048
NSPAN = (G + SPAN - 1) // SPAN  # 10

S_OUT = 2.0  # int8 full-scale in logit units (floor; adapted upward if needed)

# fp16 blob layout (element offsets): [pfT | pgT | b2q]; w2q ships as a
# separate jit arg replicated across cores (P(None)) so the runtime may
# broadcast one copy instead of uploading 8
BLOB_OFFS = (0, JH * T, JH * T + JH * U)
BLOB_N = JH * T + JH * U + V

F32 = mybir.dt.float32
F16 = mybir.dt.float16
I8 = mybir.dt.int8
ALU = mybir.AluOpType
AF = mybir.ActivationFunctionType

_CACHE = {}


def _build_program():
    nc = bacc.Bacc(None, target_bir_lowering=False)

    # per-core fp16 blob: [pfT | pgT | b2q]; w2qd replicated across cores
    blob = nc.declare_dram_parameter("blob", [1, BLOB_N], F16, isOutput=False)
    w2qd = nc.declare_dram_parameter("w2qd", [1, JH * V], F16, isOutput=False)
    outQ = nc.declare_dram_parameter("outQ", [G, V], I8, isOutput=True)
    OFF_PF, OFF_PG, OFF_B2 = BLOB_OFFS

    with tile.TileContext(nc) as tc:
        with (
            tc.tile_pool(name="const", bufs=1) as const,
            tc.tile_pool(name="hbuf", bufs=2) as hbuf,
            tc.tile_pool(name="obuf", bufs=4) as obuf,
            tc.tile_pool(name="psum", bufs=3, space="PSUM") as psum,
            tc.tile_pool(name="psumb", bufs=1, space="PSUM") as psumb,
        ):
            # ---- load inputs (small tensors first; HWDGE ring drains FIFO) ----
            pf_sb = const.tile([128, 4, T], F16, tag="pf_sb")
            nc.sync.dma_start(
                pf_sb[:],
                blob[0:1, OFF_PF : OFF_PF + JH * T].rearrange(
                    "o (c p t) -> p (o c) t", p=128, t=T
                ),
            )
            pg_sb = const.tile([128, 4, U], F16, tag="pg_sb")
            nc.sync.dma_start(
                pg_sb[:],
                blob[0:1, OFF_PG : OFF_PG + JH * U].rearrange(
                    "o (c p u) -> p (o c) u", p=128, u=U
                ),
            )
            b2q_in = const.tile([1, V], F16, tag="b2q_in")
            nc.sync.dma_start(b2q_in[:, :], blob[0:1, OFF_B2 : OFF_B2 + V])
            # w2 in two halves so jc 0-1 matmuls start at half-arrival
            w2_sb = const.tile([128, 4, V], F16, tag="w2_sb")
            for wh in range(2):
                nc.sync.dma_start(
                    w2_sb[:, 2 * wh : 2 * wh + 2, :],
                    w2qd[
                        0:1, wh * 2 * 128 * V : (wh + 1) * 2 * 128 * V
                    ].rearrange("o (c p v) -> p (o c) v", p=128, v=V),
                )

            # b2 broadcast state (emitted lazily after span-0 h-form so the
            # in-order PE queue isn't stalled on the b2q_in DMA at startup)
            b2_row = const.tile([128, V], F32, tag="b2_row")

            def emit_b2_broadcast():
                ones_sb = const.tile([1, 128], F16, tag="ones_sb")
                nc.vector.memset(ones_sb[:, :], 1.0)
                b2_ps = psumb.tile([128, V], F32, tag="b2ps")
                for vh in range(2):
                    nc.tensor.matmul(
                        b2_ps[:, vh * 512 : (vh + 1) * 512],
                        ones_sb[0:1, :],
                        b2q_in[0:1, vh * 512 : (vh + 1) * 512],
                        start=True,
                        stop=True,
                    )
                nc.vector.tensor_copy(b2_row[:, :], b2_ps[:, :])

            # ---- main loop over grid spans; short first span so the PE
            # isn't gated on a full 2048-wide h-formation at startup ----
            spans = [(0, 512), (512, 1536)]
            while spans[-1][0] + spans[-1][1] < G:
                g0n = spans[-1][0] + spans[-1][1]
                spans.append((g0n, min(SPAN, G - g0n)))
            for s, (g0, glen) in enumerate(spans):
                hts = []
                for jc in range(4):
                    ht = hbuf.tile([128, SPAN], F16, tag=f"h{jc}")
                    hts.append(ht)
                    g = g0
                    while g < g0 + glen:
                        u, t = g // T, g % T
                        seglen = min(T - t, g0 + glen - g)
                        # h = relu(pf + pg[u]) on the otherwise-idle
                        # ScalarE: out = Relu(in*1 + bias), bias per-partition
                        nc.scalar.activation(
                            ht[:, g - g0 : g - g0 + seglen],
                            pf_sb[:, jc, t : t + seglen],
                            AF.Relu,
                            bias=pg_sb[:, jc, u : u + 1],
                            scale=1.0,
                        )
                        g += seglen
                if s == 0:
                    emit_b2_broadcast()
                # grid blocks of 128 -> PSUM [g 128, vocab 1024]
                for gb0 in range(0, glen, 128):
                    gl = min(128, glen - gb0)
                    pt = psum.tile([128, V], F32, tag="pt")
                    for jc in range(4):
                        for vh in range(2):
                            nc.tensor.matmul(
                                pt[:gl, vh * 512 : (vh + 1) * 512],
                                hts[jc][:, gb0 : gb0 + gl],
                                w2_sb[:, jc, vh * 512 : (vh + 1) * 512],
                                start=(jc == 0),
                                stop=(jc == 3),
                            )
                    ob = obuf.tile([128, V], I8, tag="ob")
                    nc.vector.scalar_tensor_tensor(
                        ob[:gl, :],
                        pt[:gl, :],
                        1.0,
                        b2_row[:gl, :],
                        ALU.mult,
                        ALU.add,
                    )
                    nc.sync.dma_start(
                        outQ[g0 + gb0 : g0 + gb0 + gl, :], ob[:gl, :]
                    )

    nc.compile()
    return nc


def _get_program():
    if "nc" not in _CACHE:
        _CACHE["nc"] = _build_program()
    return _CACHE["nc"]


def _pick_scale(pfT32, pgT32, W2, b2):
    """int8 full-scale: stays at S_OUT for reference-like inputs, grows if a
    host-side sample of exactly-computed logits suggests larger magnitudes."""
    rng = np.random.default_rng(0)
    n = 3072
    bs = rng.integers(0, B, n)
    ts = rng.integers(0, T, n)
    us = rng.integers(0, U, n)
    # advanced indexing on axes 0 and 2 of (B, JH, T/U) -> (n, JH)
    h = np.maximum(pfT32[bs, :, ts] + pgT32[bs, :, us], 0.0)
    sample = h @ np.asarray(W2, dtype=np.float32).T + np.asarray(b2, np.float32)
    sample_max = float(np.abs(sample).max())
    return max(S_OUT, 1.3 * sample_max)


def _prep_inputs(f, g, W1, b1, W2, b2):
    W1f = np.asarray(W1[:, :ENC_H], dtype=np.float32)  # (JH, ENC_H)
    W1g = np.asarray(W1[:, ENC_H:], dtype=np.float32)  # (JH, PRED_H)
    f32 = np.asarray(f, dtype=np.float32)
    g32 = np.asarray(g, dtype=np.float32)
    # first layer on host (1.1% of total FLOPs, exact fp32 BLAS):
    # pfT[b] = W1f @ f[b].T  (JH, T);  pgT[b] = W1g @ g[b].T + b1  (JH, U)
    pfT32 = np.einsum("jh,bth->bjt", W1f, f32, optimize=True)
    pgT32 = np.einsum("jh,buh->bju", W1g, g32, optimize=True) + np.asarray(
        b1, dtype=np.float32
    )[None, :, None]
    scale = _pick_scale(pfT32, pgT32, W2, b2)
    qscale = 127.0 / scale
    pfT = pfT32.astype(np.float16)
    pgT = pgT32.astype(np.float16)
    W2qT = np.ascontiguousarray(np.asarray(W2, dtype=np.float32).T * qscale).astype(
        np.float16
    )  # (JH, V)
    b2qr = (np.asarray(b2, dtype=np.float32) * qscale).astype(np.float16)
    w2flat = np.ascontiguousarray(W2qT.reshape(1, -1))
    in_maps = []
    for i in range(B):
        blob = np.empty((1, BLOB_N), dtype=np.float16)
        o_pf, o_pg, o_b2 = BLOB_OFFS
        blob[0, o_pf : o_pf + JH * T] = pfT[i].reshape(-1)
        blob[0, o_pg : o_pg + JH * U] = pgT[i].reshape(-1)
        blob[0, o_b2 : o_b2 + V] = b2qr
        in_maps.append({"blob": blob, "w2qd": w2flat})
    return in_maps, scale


def _run_lean(nc, in_maps, n_cores=B):
    """PJRT shard_map dispatch with 1-element dummy output operands.

    Mirrors bass2jax.run_bass_via_pjrt, except the ExternalOutput
    pre-zero buffers are replaced by 1-element dummies: the NEFF binds
    no input to those operands (they exist so XLA *may* donate their
    buffers as pre-zeroed outputs), and this kernel writes every element
    of outQ, so uploading full-size zero buffers would be pure waste.
    """
    import jax
    from jax.sharding import Mesh, PartitionSpec
    from jax.experimental.shard_map import shard_map
    from concourse.bass2jax import (
        install_neuronx_cc_hook,
        _bass_exec_p,
        partition_id_tensor,
    )

    install_neuronx_cc_hook()

    partition_name = (
        nc.partition_id_tensor.name if nc.partition_id_tensor is not None else None
    )
    in_names, out_names, out_avals = [], [], []
    for alloc in nc.m.functions[0].allocations:
        if not isinstance(alloc, mybir.MemoryLocationSet):
            continue
        name = alloc.memorylocations[0].name
        if alloc.kind == "ExternalInput":
            if name != partition_name:
                in_names.append(name)
        elif alloc.kind == "ExternalOutput":
            out_names.append(name)
            out_avals.append(
                jax.core.ShapedArray(
                    tuple(alloc.tensor_shape), mybir.dt.np(alloc.dtype)
                )
            )

    if "lean_fn" not in _CACHE:
        all_in_names = tuple(in_names) + tuple(out_names)
        if partition_name is not None:
            all_in_names = all_in_names + (partition_name,)

        def _body(*args):
            operands = list(args)
            if partition_name is not None:
                operands.append(partition_id_tensor())
            outs = _bass_exec_p.bind(
                *operands,
                out_avals=tuple(out_avals),
                in_names=all_in_names,
                out_names=tuple(out_names),
                lowering_input_output_aliases=(),
                sim_require_finite=True,
                sim_require_nnan=True,
                nc=nc,
            )
            return tuple(outs)

        devices = jax.devices()[:n_cores]
        assert len(devices) == n_cores
        mesh = Mesh(np.asarray(devices), ("core",))
        in_specs = tuple(
            PartitionSpec() if name == "w2qd" else PartitionSpec("core")
            for name in in_names
        ) + (PartitionSpec("core"),) * len(out_names)
        out_specs = (PartitionSpec("core"),) * len(out_names)
        _CACHE["lean_fn"] = jax.jit(
            shard_map(
                _body, mesh=mesh, in_specs=in_specs, out_specs=out_specs,
                check_rep=False,
            )
        )
        _CACHE["lean_meta"] = (in_names, out_names, out_avals)

    fn = _CACHE["lean_fn"]
    in_names, out_names, out_avals = _CACHE["lean_meta"]
    concat_in = [
        np.asarray(in_maps[0][name])
        if name == "w2qd"
        else np.concatenate([np.asarray(m[name]) for m in in_maps], axis=0)
        for name in in_names
    ]
    dummies = [np.zeros((n_cores, 1), av.dtype) for av in out_avals]
    out_arrs = fn(*concat_in, *dummies)
    return [
        {
            name: np.asarray(out_arrs[i]).reshape(n_cores, *out_avals[i].shape)[c]
            for i, name in enumerate(out_names)
        }
        for c in range(n_cores)
    ]


def _assemble(results, scale_out):
    scale = np.float32(scale_out / 127.0)
    out = np.empty((B, T, U, V), dtype=np.float32)
    for i in range(B):
        oQ = results[i]["outQ"]  # (G, V) int8, grid u-major
        np.multiply(
            oQ.reshape(U, T, V).transpose(1, 0, 2),
            scale,
            out=out[i],
            casting="unsafe",
        )
    return out


def run_on_device(f, g, W1, b1, W2, b2, **spmd_kwargs):
    """Runs the kernel; returns (logits, results-or-BassKernelResults)."""
    nc = _get_program()
    in_maps, scale_out = _prep_inputs(f, g, W1, b1, W2, b2)
    if not spmd_kwargs:
        try:
            results = _run_lean(nc, in_maps)
            return _assemble(results, scale_out), None
        except Exception:
            if os.environ.get("KERNEL_LEAN_STRICT"):
                raise
    res = run_bass_kernel_spmd(nc, in_maps, list(range(B)), **spmd_kwargs)
    return _assemble(res.results, scale_out), res


def kernel(f, g, W1, b1, W2, b2):
    out, _ = run_on_device(f, g, W1, b1, W2, b2)
    return out
